# revision 9
# baseline (speedup 1.0000x reference)
"""Causal self-attention (RoPE) Trainium2 kernel.

Full-input contract: kernel(**inputs) takes the unsharded tensors
(x[2,2048,1024], Wq/Wk/Wv/Wp[1024,1024], bq/bk/bv/bp[1024]) and returns the
full [2,2048,1024] output.

Sharding (8 cores): data-parallel over batch (2) x tensor-parallel over heads
(4 groups of 4 heads = 256 channels). Core i handles batch i//4, head-group
i%4. Each core computes a [2048,1024] partial of the output projection; the
host sums the 4 TP partials per batch and adds the (bv-folded) projection
bias.

Per-core kernel design (all matmuls in float32r = full-rate FP22):
  - Host pre-transposes x to xT[c,t] (t-tile-major, flat [128,4096] DMA
    blocks) and weights to WqT/WkT/WvT[c,m] so q/k are produced directly in
    [head_dim, t] layout (natural for scores) and v in [t, head_dim] layout
    (natural as the att@v stationary operand). All DMAs are 2D-contiguous
    [128, N] APs (3D APs don't collapse -> descriptor-bound DMA).
  - RoPE in [d,t] layout: pair-swap via DVE stream_shuffle (PSUM input) +
    two scalar_tensor_tensor fmas with host-precomputed cos/(+-)sin tiles;
    final add on GPSIMD for engine balance.
  - scores^T[tk,tq] = kT.T @ qT, two heads paired at PE row groups 0/64
    (K=64 matmuls at the same row group serialize ~3.6x slower).
  - exp on ACT (scale=1/8 folded, no max-subtraction: logits are O(5));
    causal mask via gpsimd affine_select (fill=0 after exp) on diagonal
    blocks only; fully-masked blocks skipped.
  - att@v with V augmented by a ones column -> row 64 of the accumulator is
    the softmax denominator; normalize via DVE reciprocal (cross-quadrant
    1-partition write) + gpsimd partition_broadcast + DVE multiply.
  - output projection from yT tiles against host-pre-transposed WpT, K=128
    accumulation, natural [t,c] output assembled in SBUF and stored as
    [128, 4KB-contiguous] rows.
  - bv never enters the kernel: softmax rows sum to 1, so att@(v+bv) =
    att@v + bv; bv's contribution is folded into bp on the host
    (bp_eff = bp + Wp @ bv).
"""

import numpy as np

B, T, C = 2, 2048, 1024
NHEAD_TOT = 16
HS = 64
NCORE = 8
TPG = 4  # tensor-parallel group size (head groups)
HD = C // TPG  # 256 channels per core
NHC = HD // HS  # 4 heads per core
TT = 512  # t-tile (matmul free dim)
NT = T // TT  # 4
KB = C // 128  # 8 contraction blocks over C

_COMPILED = None


def _swap_mask():
    # stream_shuffle mask: swap adjacent partitions within each 32-group
    return [i ^ 1 for i in range(32)]


def _build_kernel_body(ctx, tc, out, xt, wqt, wkt, wvt, wpt, bias, trig):
    import concourse.mybir as mybir
    from concourse.bass import ts

    nc = tc.nc
    F32 = mybir.dt.float32
    F32R = mybir.dt.float32r
    EXP = mybir.ActivationFunctionType.Exp
    MUL = mybir.AluOpType.mult
    ADD = mybir.AluOpType.add
    GE = mybir.AluOpType.is_ge
    SWAP = _swap_mask()

    const = ctx.enter_context(tc.tile_pool(name="const", bufs=1))
    xpool = ctx.enter_context(tc.tile_pool(name="xpool", bufs=2))
    resid = ctx.enter_context(tc.tile_pool(name="resid", bufs=1))
    ropet = ctx.enter_context(tc.tile_pool(name="ropet", bufs=3))
    expp = ctx.enter_context(tc.tile_pool(name="expp", bufs=4))
    smallp = ctx.enter_context(tc.tile_pool(name="smallp", bufs=2))
    outp = ctx.enter_context(tc.tile_pool(name="outp", bufs=2))
    mm_ps = ctx.enter_context(tc.tile_pool(name="mm_ps", bufs=2, space="PSUM"))
    st_ps = ctx.enter_context(tc.tile_pool(name="st_ps", bufs=2, space="PSUM"))
    acc_ps = ctx.enter_context(tc.tile_pool(name="acc_ps", bufs=1, space="PSUM"))

    # ---- constants (weights/trig on gpsimd SWDGE; x stream stays on sync) ----
    wq_s = const.tile([128, KB * HD], F32R)
    wk_s = const.tile([128, KB * HD], F32R)
    wv_s = const.tile([128, KB * HD], F32R)
    wp_s = const.tile([128, 2 * C], F32R)
    cos_s = const.tile([128, T], F32)
    sin_s = const.tile([128, T], F32)
    nc.gpsimd.dma_start(out=wq_s, in_=wqt)
    nc.gpsimd.dma_start(out=wk_s, in_=wkt)
    nc.gpsimd.dma_start(out=wv_s, in_=wvt)
    nc.gpsimd.dma_start(out=wp_s, in_=wpt)
    nc.gpsimd.dma_start(out=cos_s, in_=trig[0])
    nc.gpsimd.dma_start(out=sin_s, in_=trig[1])
    # bias[qk, mb, raw/swapped, 128, 1]
    b_tiles = {}
    for qk in range(2):
        for mb in range(2):
            for sw in range(2):
                t = const.tile([128, 1], F32, tag=f"b{qk}{mb}{sw}")
                nc.sync.dma_start(out=t, in_=bias[qk, mb, sw])
                b_tiles[(qk, mb, sw)] = t

    # ---- resident activations ----
    qTr = resid.tile([128, 2, T], F32R)  # roped q, [d, mb, t]
    kTr = resid.tile([128, 2, T], F32R)  # roped k
    v_s = resid.tile([128, 16, NHC, HS + 1], F32R)  # [tk-part, tk-blk, h, 64+1s]
    yT = resid.tile([128, 2, T], F32R)  # attention out (normalized), [d, db, t]
    ones_f = const.tile([128, 16 * NHC], F32)
    nc.vector.memset(ones_f, 1.0)
    nc.vector.tensor_copy(
        out=v_s[:, :, :, HS : HS + 1],
        in_=ones_f.rearrange("p (a b c) -> p a b c", a=16, b=NHC),
    )

    def qkv_phase(tt):
        x_t = xpool.tile([128, KB * TT], F32R)
        nc.sync.dma_start(out=x_t, in_=xt[tt])

        # q and k projections (transposed layout) + RoPE
        for qk, (w_s, dest) in enumerate(((wq_s, qTr), (wk_s, kTr))):
            for mb in range(2):
                ps = mm_ps.tile([128, TT], F32, tag="mm")
                for kb in range(KB):
                    nc.tensor.matmul(
                        ps,
                        lhsT=w_s[:, kb * HD + mb * 128 : kb * HD + (mb + 1) * 128],
                        rhs=x_t[:, ts(kb, TT)],
                        start=(kb == 0),
                        stop=(kb == KB - 1),
                    )
                # RoPE: dest = (ps+b)*cos + (swap(ps)+swap(b))*sinhat
                shuf = ropet.tile([128, TT], F32, tag="shuf")
                nc.vector.stream_shuffle(shuf, ps, SWAP)
                t1 = ropet.tile([128, TT], F32, tag="t1")
                nc.vector.scalar_tensor_tensor(
                    out=t1, in0=ps, scalar=b_tiles[(qk, mb, 0)],
                    in1=cos_s[:, ts(tt, TT)], op0=ADD, op1=MUL,
                )
                nc.vector.scalar_tensor_tensor(
                    out=shuf, in0=shuf, scalar=b_tiles[(qk, mb, 1)],
                    in1=sin_s[:, ts(tt, TT)], op0=ADD, op1=MUL,
                )
                nc.gpsimd.tensor_tensor(dest[:, mb, ts(tt, TT)], t1, shuf, ADD)

        # v projection (natural layout), no bias (folded into bp on host)
        for tsub in range(TT // 128):
            ps = mm_ps.tile([128, HD], F32, tag="mm")
            for kb in range(KB):
                nc.tensor.matmul(
                    ps,
                    lhsT=x_t[:, kb * TT + tsub * 128 : kb * TT + (tsub + 1) * 128],
                    rhs=wv_s[:, ts(kb, HD)],
                    start=(kb == 0),
                    stop=(kb == KB - 1),
                )
            blk = tt * (TT // 128) + tsub
            nc.vector.tensor_copy(
                out=v_s[:, blk, :, 0:HS],
                in_=ps.rearrange("p (h e) -> p h e", h=NHC),
            )

    def attn_phase(qt):
        nkt = (qt + 1) * (TT // 128)
        for hp in range(NHC // 2):
            heads = (2 * hp, 2 * hp + 1)  # partition bases 0 / 64
            mb = hp
            oaccs = {}
            for h in heads:
                oacc = acc_ps.tile([HS + 1, TT], F32, tag=f"oacc{h % 2}",
                                   name=f"oacc_{qt}_{h}")
                oaccs[h] = oacc
            for kt in range(nkt):
                sts = {}
                for h in heads:
                    poff = (h % 2) * 64
                    st = st_ps.tile([128, TT], F32, tag=f"st{h % 2}",
                                    name=f"st_{qt}_{h}_{kt}")
                    # paired K=64 matmuls at row groups 0/64 run concurrently
                    nc.tensor.matmul(
                        st,
                        lhsT=kTr[poff : poff + 64, mb, ts(kt, 128)],
                        rhs=qTr[poff : poff + 64, mb, ts(qt, TT)],
                        start=True,
                        stop=True,
                    )
                    sts[h] = st
                r = kt * 128 - qt * TT
                for h in heads:
                    ex = expp.tile([128, TT], F32R, tag="ex")
                    nc.scalar.activation(ex, sts[h], EXP, scale=0.125)
                    if r >= 0:  # diagonal block: keep j >= i + r, else 0
                        nc.gpsimd.affine_select(
                            out=ex, in_=ex, pattern=[[1, TT]], base=-r,
                            channel_multiplier=-1, compare_op=GE, fill=0.0,
                        )
                    nc.tensor.matmul(
                        oaccs[h],
                        lhsT=v_s[:, kt, h, :],
                        rhs=ex,
                        start=(kt == 0),
                        stop=(kt == nkt - 1),
                    )
            # normalize: yT[poff:poff+64, mb, qt] = oacc[0:64] / oacc[64]
            for h in heads:
                poff = (h % 2) * 64
                oacc = oaccs[h]
                rden = smallp.tile([128, TT], F32, tag="rden")
                nc.vector.reciprocal(rden[0:1, :], oacc[64:65, :])
                bden = smallp.tile([64, TT], F32, tag="bden")
                nc.gpsimd.partition_broadcast(bden, rden[0:1, :])
                nc.vector.tensor_tensor(
                    yT[poff : poff + 64, mb, ts(qt, TT)], oacc[0:64, :], bden, MUL
                )

    def proj_phase(qt):
        for tsub in range(TT // 128):
            tch = qt * (TT // 128) + tsub
            osb = outp.tile([128, C], F32)
            for cb in range(C // TT):
                ps = mm_ps.tile([128, TT], F32, tag="mm")
                for db in range(2):
                    nc.tensor.matmul(
                        ps,
                        lhsT=yT[:, db, ts(tch, 128)],
                        rhs=wp_s[:, db * C + cb * TT : db * C + (cb + 1) * TT],
                        start=(db == 0),
                        stop=(db == 1),
                    )
                nc.vector.tensor_copy(out=osb[:, ts(cb, TT)], in_=ps[:])
            nc.scalar.dma_start(out=out[ts(tch, 128), :], in_=osb)

    # interleave so ACT-paced attention overlaps PE-dense qkv/proj
    qkv_phase(0)
    attn_phase(0)
    qkv_phase(1)
    proj_phase(0)
    attn_phase(1)
    qkv_phase(2)
    proj_phase(1)
    attn_phase(2)
    qkv_phase(3)
    proj_phase(2)
    attn_phase(3)
    proj_phase(3)


def build_module(loop_n=1):
    from contextlib import ExitStack

    import concourse.bacc as bacc
    import concourse.mybir as mybir
    import concourse.tile as tile

    F32 = mybir.dt.float32
    F32R = mybir.dt.float32r
    nc = bacc.Bacc(
        "TRN2", target_bir_lowering=False, debug=False, num_devices=NCORE
    )
    xt = nc.dram_tensor("xt", [NT, 128, KB * TT], F32R, kind="ExternalInput").ap()
    wqt = nc.dram_tensor("wqt", [128, KB * HD], F32R, kind="ExternalInput").ap()
    wkt = nc.dram_tensor("wkt", [128, KB * HD], F32R, kind="ExternalInput").ap()
    wvt = nc.dram_tensor("wvt", [128, KB * HD], F32R, kind="ExternalInput").ap()
    wpt = nc.dram_tensor("wpt", [128, 2 * C], F32R, kind="ExternalInput").ap()
    bias = nc.dram_tensor("bias", [2, 2, 2, 128, 1], F32, kind="ExternalInput").ap()
    trig = nc.dram_tensor("trig", [2, 128, T], F32, kind="ExternalInput").ap()
    out = nc.dram_tensor("out", [T, C], F32, kind="ExternalOutput").ap()

    with tile.TileContext(nc) as tc:
        with ExitStack() as ctx:
            if loop_n == 1:
                _build_kernel_body(ctx, tc, out, xt, wqt, wkt, wvt, wpt, bias, trig)
            else:
                with tc.For_i(0, loop_n, 1):
                    _build_kernel_body(
                        ctx, tc, out, xt, wqt, wkt, wvt, wpt, bias, trig
                    )
    nc.compile()
    return nc


def _get_compiled():
    global _COMPILED
    if _COMPILED is None:
        _COMPILED = build_module()
    return _COMPILED


def make_in_maps(x, Wq, bq, Wk, bk, Wv, bv, Wp, bp):
    x = np.asarray(x, dtype=np.float32)
    Wq = np.asarray(Wq, dtype=np.float32)
    Wk = np.asarray(Wk, dtype=np.float32)
    Wv = np.asarray(Wv, dtype=np.float32)
    Wp = np.asarray(Wp, dtype=np.float32)
    bq = np.asarray(bq, dtype=np.float32)
    bk = np.asarray(bk, dtype=np.float32)

    # trig tiles: row d -> freq index (d%64)//2; sinhat sign: -1 on even rows
    i_idx = (np.arange(128) % HS) // 2
    theta = (1.0 / (10000.0 ** (np.arange(0, HS, 2, dtype=np.float32) / HS))).astype(
        np.float32
    )
    tpos = np.arange(T, dtype=np.float32)
    freqs = tpos[:, None] * theta[None, :]  # [T, 32] fp32 like reference
    cosf = np.cos(freqs).astype(np.float32)  # [T, 32]
    sinf = np.sin(freqs).astype(np.float32)
    sign = np.where(np.arange(128) % 2 == 0, -1.0, 1.0).astype(np.float32)
    trig = np.empty((2, 128, T), dtype=np.float32)
    trig[0] = cosf[:, i_idx].T
    trig[1] = sinf[:, i_idx].T * sign[:, None]
    trig = np.ascontiguousarray(trig)

    swap_pairs = np.arange(256) ^ 1

    def part_major_flat(a, nblk):
        """[nblk*128, m] -> [128, nblk*m] (kb-major per partition)."""
        n, m = a.shape
        assert n == nblk * 128
        return np.ascontiguousarray(
            a.reshape(nblk, 128, m).transpose(1, 0, 2).reshape(128, nblk * m)
        )

    in_maps = []
    for core in range(NCORE):
        b, hg = core // TPG, core % TPG
        sl = slice(hg * HD, (hg + 1) * HD)
        # xt[tt, p, kb*512+j] = x[b, tt*512+j, kb*128+p]
        xb = x[b].T.reshape(KB, 128, NT, TT)  # [kb, p, tt, j]
        xtile = np.ascontiguousarray(
            xb.transpose(2, 1, 0, 3).reshape(NT, 128, KB * TT)
        )
        wqt = part_major_flat(np.ascontiguousarray(Wq[sl, :].T), KB)
        wkt = part_major_flat(np.ascontiguousarray(Wk[sl, :].T), KB)
        wvt = part_major_flat(np.ascontiguousarray(Wv[sl, :].T), KB)
        wpt = part_major_flat(np.ascontiguousarray(Wp[:, sl].T), 2)
        bias = np.empty((2, 2, 2, 128, 1), dtype=np.float32)
        for qk, bvec in enumerate((bq[sl], bk[sl])):
            bsw = bvec[swap_pairs]
            for mb in range(2):
                bias[qk, mb, 0, :, 0] = bvec[mb * 128 : (mb + 1) * 128]
                bias[qk, mb, 1, :, 0] = bsw[mb * 128 : (mb + 1) * 128]
        in_maps.append(
            {
                "xt": xtile,
                "wqt": wqt,
                "wkt": wkt,
                "wvt": wvt,
                "wpt": wpt,
                "bias": bias,
                "trig": trig,
            }
        )
    return in_maps


def assemble_output(results, Wp, bp, bv):
    Wp = np.asarray(Wp, dtype=np.float32)
    bp = np.asarray(bp, dtype=np.float32)
    bv = np.asarray(bv, dtype=np.float32)
    bp_eff = bp + Wp @ bv
    out = np.zeros((B, T, C), dtype=np.float32)
    for core in range(NCORE):
        b = core // TPG
        out[b] += results[core]["out"]
    out += bp_eff[None, None, :]
    return out


def run(trace=False, **inputs):
    from concourse.bass_utils import run_bass_kernel_spmd

    nc = _get_compiled()
    in_maps = make_in_maps(**inputs)
    kw = {}
    if trace:
        kw = dict(trace=True, trace_cores=list(range(NCORE)))
    res = run_bass_kernel_spmd(nc, in_maps, list(range(NCORE)), **kw)
    out = assemble_output(res.results, inputs["Wp"], inputs["bp"], inputs["bv"])
    return out, res


def kernel(**inputs):
    out, _ = run(trace=False, **inputs)
    return out
